# revision 2
# baseline (speedup 1.0000x reference)
"""Distributed AstrometryConcordanceHead kernel for 8 Trainium2 NeuronCores.

Pure data parallel: batch B=8 sharded one sample per NeuronCore; the tiny
params (Wr, Wv, log_temperature) are replicated. Each core computes its
projection, local cost volume (R=3 -> K=49 shifts) and soft-argmax
independently; outputs are gathered to the full [8, 5, 192, 192] result.
"""

import numpy as np

R = 3
K = (2 * R + 1) ** 2
EPS_NORM = 1e-6
TAU_MIN = 1e-3

# Hardcoded problem shapes (self-contained; no spec.json reads).
B, D, H, W = 8, 256, 192, 192
Dm = 64

_COMPILED = {}


def _build():
    import jax
    import jax.numpy as jnp
    from functools import partial

    devs = jax.devices()[:8]

    offs = np.arange(-R, R + 1, dtype=np.float32)
    dy_lut = np.repeat(offs, 2 * R + 1).reshape(1, K, 1, 1)
    dx_lut = np.tile(offs, 2 * R + 1).reshape(1, K, 1, 1)

    def per_core(rubin_2d, vis_2d, Wr, Wv, log_temperature):
        # rubin_2d/vis_2d: [1, D, H, W] on this core
        b, d, h, w = rubin_2d.shape
        # 1x1 projections: [1, Dm, H, W]
        rub = jnp.einsum('bdhw,md->bmhw', rubin_2d, Wr)
        vis = jnp.einsum('bdhw,md->bmhw', vis_2d, Wv)

        def l2n(x):
            n = jnp.sqrt(jnp.sum(x * x, axis=1, keepdims=True))
            return x / jnp.maximum(n, EPS_NORM)

        rub_n = l2n(rub)
        vis_n = l2n(vis)
        vis_pad = jnp.pad(vis_n, ((0, 0), (0, 0), (R, R), (R, R)), mode='edge')
        scale = 1.0 / np.sqrt(float(Dm))

        corrs = []
        for dy in range(-R, R + 1):
            for dx in range(-R, R + 1):
                shifted = vis_pad[:, :, R + dy:R + dy + h, R + dx:R + dx + w]
                corrs.append(jnp.sum(rub_n * shifted, axis=1) * scale)
        logits = jnp.stack(corrs, axis=1)  # [1, K, H, W]

        dy_l = jnp.asarray(dy_lut)
        dx_l = jnp.asarray(dx_lut)

        tau = jnp.maximum(jnp.exp(log_temperature[0]), TAU_MIN)
        probs_local = jax.nn.softmax(logits / tau, axis=1)
        dy_local = jnp.sum(probs_local * dy_l, axis=1, keepdims=True)
        dx_local = jnp.sum(probs_local * dx_l, axis=1, keepdims=True)
        conf_local = jnp.max(probs_local, axis=1, keepdims=True)

        logits_global = jnp.mean(logits, axis=(2, 3), keepdims=True)
        probs_global = jax.nn.softmax(logits_global / tau, axis=1)
        dy_global = jnp.sum(probs_global * dy_l, axis=1, keepdims=True)
        dx_global = jnp.sum(probs_global * dx_l, axis=1, keepdims=True)
        conf_global = jnp.max(probs_global, axis=1, keepdims=True)

        uniform = 1.0 / float(K)
        lw = jnp.clip((conf_local - uniform) / max(1e-6, 1.0 - uniform), 0.0, 1.0)
        dy_o = lw * dy_local + (1.0 - lw) * dy_global
        dx_o = lw * dx_local + (1.0 - lw) * dx_global
        conf_g = jnp.broadcast_to(conf_global, (b, 1, h, w))
        return jnp.concatenate([dy_o, dx_o, conf_local, lw, conf_g], axis=1)

    fn = jax.pmap(
        per_core,
        in_axes=(0, 0, None, None, None),
        devices=devs,
    )
    return fn


def kernel(rubin_2d, vis_2d, Wr, Wv, log_temperature):
    if 'fn' not in _COMPILED:
        _COMPILED['fn'] = _build()
    fn = _COMPILED['fn']

    rubin = np.asarray(rubin_2d, dtype=np.float32).reshape(B, 1, D, H, W)
    vis = np.asarray(vis_2d, dtype=np.float32).reshape(B, 1, D, H, W)
    Wr = np.asarray(Wr, dtype=np.float32)
    Wv = np.asarray(Wv, dtype=np.float32)
    lt = np.asarray(log_temperature, dtype=np.float32)

    out = fn(rubin, vis, Wr, Wv, lt)  # [8, 1, 5, H, W]
    out = np.asarray(out).reshape(B, 5, H, W).astype(np.float32)
    return out
